# revision 22
# baseline (speedup 1.0000x reference)
"""MMD loss kernel for Trainium2 (8 NeuronCores, Bass/Tile).

Math: out = mean_k mean_ij exp(-c_k * ||x_i - x_j||^2)          (kss)
          + same for y                                          (ktt)
          - 2 * same for (x, y)                                 (kst)
      with c_k = 1/(2 b_k^2), x: [8192, 256], y: [8192, 256].

Device strategy (identical SPMD program on 8 cores, different data):
  * PE computes the pairwise squared distances directly via feature
    augmentation: dist = [-2x; nh; nl; 1; 1]^T . [y; 1; 1; nh; nl]
    in bf16 (fp32 PSUM accumulate), K = 256 + 4.
  * ScalarE evaluates exp(-c_k * d) straight from PSUM in [128, 2048]
    chunks with fused accum_out row-sums (the mean reduction is free).
  * kss/ktt use a symmetric band decomposition: each 128-row tile r
    covers col tiles r+1..r+32 (mod 64) with weight 2, a d=32 batch
    with weight -1 removes the double count, and the diagonal subtiles
    (weight +1) have their exact diagonal masked to +1e30 (exp -> 0);
    the true diagonal contribution (N*K per matrix) is added on the
    host analytically.  This removes 1/3 of the exp work.
  * Per-core work: row tiles {8j + core}.  A per-core column rotation
    by 128*(core+1) makes every access offset core-independent, so one
    NEFF serves all 8 cores.

Fast path: the kernel-mean sum for bandwidth k is Sigma exp(-c_k d).
For a_j = exp(-c0 d_j) >= 0 and p = c_k/c0 >= 1, the lp-in-l1 norm
inequality gives Sigma a_j^p <= (Sigma a_j)^p per (row, chunk).  So the
device only evaluates exp for c0 = min_k c_k (one ACT pass per chunk);
the host bounds every other bandwidth's off-diagonal contribution from
the per-row-chunk partial sums the kernel already produces.  When the
bound is not negligible (never for gaussian-scale data, where larger
c_k underflow fp32 anyway) it falls back to the general multi-exp
kernel, so the result is correct for arbitrary inputs.
"""

import os
import numpy as np
import ml_dtypes

import concourse.bass as bass
import concourse.mybir as mybir
import concourse.tile as tile
from concourse import bacc
from concourse.bass_utils import run_bass_kernel_spmd

bf16 = ml_dtypes.bfloat16

N, D, P = 8192, 256, 128
NCORES, JPC = 8, 8          # 64 row tiles of 128, 8 per core
CHUNK = 2048                # PSUM chunk (4 banks) / ACT free dim
BANK = 512
NT = N // P                 # 64 subtile columns
BIG = np.float32(1e30)

# ---------------------------------------------------------------- job list


def chunk_list():
    """Chunk descriptors, identical on every core.

    (kind, lhs_tile, rhs_role, rhs_start, weight)
      kind: 'mm' (12-matmul streaming chunk) or 'sub16' (16 subtiles)
    """
    chunks = []
    # kst column-major: the 8 jobs of column piece cb only need that piece
    # of ry, so compute starts as soon as the first ~1 MB of DMA lands.
    for cb in range(4):
        for j in range(JPC):                  # kst, weight -2
            chunks.append(("mm", j, "y", cb * CHUNK, -2.0, False))
    for j in range(JPC):                      # kss band, weight +2
        for cb in range(2):
            chunks.append(("mm", j, "x", (1024 * j + CHUNK * cb) % N, 2.0, False))
    # the sub16 specials sit mid-stream so the kernel tail stays on the
    # regular pipeline
    chunks.append(("sub16", None, None, "d32", -1.0, False))   # d=32 fix
    chunks.append(("sub16", None, None, "diag", 1.0, True))    # masked diag
    for j in range(JPC):                      # ktt band, weight +2
        for cb in range(2):
            chunks.append(("mm", 8 + j, "y", (1024 * j + CHUNK * cb) % N, 2.0, False))
    return chunks


def sub16_layout(batch):
    """16 (lhs_tile, role, rhs_start) triples for a sub16 chunk."""
    out = []
    for s in range(16):
        jj = s % 8
        role = "x" if s < 8 else "y"
        if batch == "d32":
            st = (1024 * jj + 3968) % N
        else:
            st = (1024 * jj - 128) % N
        out.append((s, role, st))
    return out


NCHUNKS = len(chunk_list())  # 66

# ---------------------------------------------------------------- device


def pick_split(cs):
    """Find power-of-4 chains so some exp terms move to VectorE.

    Returns (base_idx, pow4_idx, pow16_idx|None) or None.  For the
    canonical bandwidths [0.1, 0.5, 1, 2, 5] -> cs = [50, 2, .5, .125,
    .02]: base c=0.125 (b=2), offloaded c=0.5 = base^4 and c=2 = base^16.
    """
    K = len(cs)

    def near(a, b):
        return abs(a - b) <= 1e-6 * abs(b)

    best = None
    for i in range(K):
        for j in range(K):
            if i == j or not near(cs[j], 4.0 * cs[i]):
                continue
            if best is None:
                best = (i, j, None)
            for k in range(K):
                if k not in (i, j) and near(cs[k], 4.0 * cs[j]):
                    return (i, j, k)
    return best


GENERAL_KEYS = (
    "lhs0", "lhs1", "laug", "rx0", "rx1", "ry0", "ry1",
    "raugx", "raugy", "maskd",
)
FAST_KEYS = (
    "lhs0", "lhs1", "laug", "rx0", "rx1", "ry0", "ry1",
    "raugx", "raugy", "xnb", "ynb", "normb",
)
NPAD = N + CHUNK  # broadcast norm rows padded for wrapping windows
DIAG_Q = 49  # chunk index of the unmasked diagonal-block chunk


def _declare_io(nc, acc_cols):
    f32, b16 = mybir.dt.float32, mybir.dt.bfloat16
    d = {}
    d["lhs0"] = nc.dram_tensor("lhs0", [P, 16 * P], b16, kind="ExternalInput").ap()
    d["lhs1"] = nc.dram_tensor("lhs1", [P, 16 * P], b16, kind="ExternalInput").ap()
    d["laug"] = nc.dram_tensor("laug", [4, 16 * P], b16, kind="ExternalInput").ap()
    d["rx0"] = nc.dram_tensor("rx0", [P, N], b16, kind="ExternalInput").ap()
    d["rx1"] = nc.dram_tensor("rx1", [P, N], b16, kind="ExternalInput").ap()
    d["ry0"] = nc.dram_tensor("ry0", [P, N], b16, kind="ExternalInput").ap()
    d["ry1"] = nc.dram_tensor("ry1", [P, N], b16, kind="ExternalInput").ap()
    d["raugx"] = nc.dram_tensor("raugx", [4, N], b16, kind="ExternalInput").ap()
    d["raugy"] = nc.dram_tensor("raugy", [4, N], b16, kind="ExternalInput").ap()
    d["maskd"] = nc.dram_tensor("maskd", [P, CHUNK], b16, kind="ExternalInput").ap()
    d["acc"] = nc.dram_tensor("acc", [P, acc_cols], f32, kind="ExternalOutput").ap()
    return d


def _alloc_and_dma(nc, consts, dio, acc_cols):
    """Allocate SBUF const tiles and issue the input DMAs in the order
    the chunk stream consumes them (kst piece 0 first)."""
    f32, b16 = mybir.dt.float32, mybir.dt.bfloat16
    t = {}
    t["lhs0"] = consts.tile([P, 16 * P], b16, name="lhs0")
    t["lhs1"] = consts.tile([P, 16 * P], b16, name="lhs1")
    t["laug"] = consts.tile([4, 16 * P], b16, name="laug")
    t["rx0"] = consts.tile([P, N], b16, name="rx0")
    t["rx1"] = consts.tile([P, N], b16, name="rx1")
    t["ry0"] = consts.tile([P, N], b16, name="ry0")
    t["ry1"] = consts.tile([P, N], b16, name="ry1")
    t["raugx"] = consts.tile([4, N], b16, name="raugx")
    t["raugy"] = consts.tile([4, N], b16, name="raugy")
    t["maskd"] = consts.tile([P, CHUNK], b16, name="maskd")
    t["acc"] = consts.tile([P, acc_cols], f32, name="acc")

    nc.vector.memset(t["acc"], 0.0)
    half = 8 * P
    for k in ("lhs0", "lhs1", "laug"):
        nc.sync.dma_start(out=t[k][:, :half], in_=dio[k][:, :half])
    nc.sync.dma_start(out=t["raugy"], in_=dio["raugy"])
    for k in ("lhs0", "lhs1", "laug"):
        nc.sync.dma_start(out=t[k][:, half:], in_=dio[k][:, half:])
    nc.sync.dma_start(out=t["raugx"], in_=dio["raugx"])
    for piece in range(4):
        csl = slice(CHUNK * piece, CHUNK * (piece + 1))
        for k in ("ry0", "ry1"):
            nc.sync.dma_start(out=t[k][:, csl], in_=dio[k][:, csl])
    for piece in range(4):
        csl = slice(CHUNK * piece, CHUNK * (piece + 1))
        for k in ("rx0", "rx1"):
            nc.sync.dma_start(out=t[k][:, csl], in_=dio[k][:, csl])
    nc.sync.dma_start(out=t["maskd"], in_=dio["maskd"])
    return t


def _chunk_jobs(kind, t, start):
    if kind == "mm":
        return [
            (BANK * b, BANK, t, None, (start + BANK * b) % N)
            for b in range(4)
        ]
    return [
        (P * s16, P, s16, role2, st2)
        for (s16, role2, st2) in sub16_layout(start)
    ]


def _emit_chunk_mms(nc, tiles, psum, role, jobs, n_ki=3):
    """jobs: list of (pcol, width, lhs_tile, role_override, rhs_start).
    k-outer / job-inner order so each lhsT loads once per contraction
    slice instead of once per bank.  n_ki=2 skips the augmentation pass
    (norms added later on VectorE instead)."""
    rmain = {"x": (tiles["rx0"], tiles["rx1"]), "y": (tiles["ry0"], tiles["ry1"])}
    raug_t = {"x": tiles["raugx"], "y": tiles["raugy"]}
    for ki in range(n_ki):
        for (pcol, width, t, role_ov, start) in jobs:
            r = role_ov or role
            m0, m1 = rmain[r]
            lsl = slice(P * t, P * t + P)
            if ki == 0:
                l, rr = tiles["lhs0"][:, lsl], m0[:, start : start + width]
            elif ki == 1:
                l, rr = tiles["lhs1"][:, lsl], m1[:, start : start + width]
            else:
                l, rr = tiles["laug"][:, lsl], raug_t[r][:, start : start + width]
            # start=True clears the has_written bits of the WHOLE psum
            # bank, so only the first matmul touching each bank carries it
            # (sub16 jobs pack four 128-col subtiles per 512-col bank).
            nc.tensor.matmul(
                psum[:, pcol : pcol + width], l, rr,
                start=(ki == 0 and pcol % BANK == 0), stop=(ki == n_ki - 1),
            )


def _declare_io_fast(nc):
    f32, b16 = mybir.dt.float32, mybir.dt.bfloat16
    d = {}
    d["lhs0"] = nc.dram_tensor("lhs0", [P, 16 * P], b16, kind="ExternalInput").ap()
    d["lhs1"] = nc.dram_tensor("lhs1", [P, 16 * P], b16, kind="ExternalInput").ap()
    d["laug"] = nc.dram_tensor("laug", [4, 16 * P], b16, kind="ExternalInput").ap()
    d["rx0"] = nc.dram_tensor("rx0", [P, N], b16, kind="ExternalInput").ap()
    d["rx1"] = nc.dram_tensor("rx1", [P, N], b16, kind="ExternalInput").ap()
    d["ry0"] = nc.dram_tensor("ry0", [P, N], b16, kind="ExternalInput").ap()
    d["ry1"] = nc.dram_tensor("ry1", [P, N], b16, kind="ExternalInput").ap()
    d["raugx"] = nc.dram_tensor("raugx", [4, N], b16, kind="ExternalInput").ap()
    d["raugy"] = nc.dram_tensor("raugy", [4, N], b16, kind="ExternalInput").ap()
    d["xnb"] = nc.dram_tensor("xnb", [P, NPAD], b16, kind="ExternalInput").ap()
    d["ynb"] = nc.dram_tensor("ynb", [P, NPAD], b16, kind="ExternalInput").ap()
    d["normb"] = nc.dram_tensor("normb", [P, 16], f32, kind="ExternalInput").ap()
    d["acc"] = nc.dram_tensor("acc", [P, NCHUNKS], f32, kind="ExternalOutput").ap()
    return d


def build_fast_kernel(neg_c0):
    """Single-bandwidth NEFF.  Per [128, 2048] chunk (pipelined):
      * PE: two bf16 128-deep contraction passes (8 matmuls, ~1.8us);
        the first kst batch and the sub16 specials also run the rank-4
        norm augmentation pass on PE (12 matmuls).
      * VectorE (most chunks): one scalar_tensor_tensor adds both norms,
        (psum + xn_p) + ynb_j -> f32 staging tile, ~2.3us.
      * ScalarE: ONE exp with fused accum_out row-sum into acc[:, q],
        ~2.1us.
    The diagonal subtiles are computed unmasked (d_ii ~ 0 -> exp ~ 1);
    the host subtracts the 2N near-unit diagonal terms analytically."""
    nc = bacc.Bacc("TRN2", debug=False, enable_asserts=False, num_devices=NCORES)
    f32, b16 = mybir.dt.float32, mybir.dt.bfloat16
    dio = _declare_io_fast(nc)

    with tile.TileContext(nc) as tc:
        with (
            tc.tile_pool(name="consts", bufs=1) as consts,
            tc.tile_pool(name="scr", bufs=2) as scrp,
            tc.tile_pool(name="psum", bufs=2, space="PSUM") as psump,
        ):
            t = {}
            t["lhs0"] = consts.tile([P, 16 * P], b16, name="lhs0")
            t["lhs1"] = consts.tile([P, 16 * P], b16, name="lhs1")
            t["laug"] = consts.tile([4, 16 * P], b16, name="laug")
            t["rx0"] = consts.tile([P, N], b16, name="rx0")
            t["rx1"] = consts.tile([P, N], b16, name="rx1")
            t["ry0"] = consts.tile([P, N], b16, name="ry0")
            t["ry1"] = consts.tile([P, N], b16, name="ry1")
            t["raugx"] = consts.tile([4, N], b16, name="raugx")
            t["raugy"] = consts.tile([4, N], b16, name="raugy")
            t["xnb"] = consts.tile([P, NPAD], b16, name="xnb")
            t["ynb"] = consts.tile([P, NPAD], b16, name="ynb")
            t["normb"] = consts.tile([P, 16], f32, name="normb")
            acc = consts.tile([P, NCHUNKS], f32, name="acc")
            nc.vector.memset(acc, 0.0)

            half = 8 * P

            def dma(key, sl=None):
                if sl is None:
                    nc.sync.dma_start(out=t[key], in_=dio[key])
                else:
                    nc.sync.dma_start(out=t[key][:, sl], in_=dio[key][:, sl])

            # DMA stream in chunk-consumption order
            for k in ("lhs0", "lhs1", "laug"):
                dma(k, slice(0, half))
            dma("raugy")
            dma("ry0", slice(0, CHUNK))
            dma("ry1", slice(0, CHUNK))          # <- chunks 0-7 ready
            dma("normb")
            dma("ry0", slice(CHUNK, 2 * CHUNK))
            dma("ry1", slice(CHUNK, 2 * CHUNK))
            dma("ynb", slice(0, CHUNK))
            dma("ynb", slice(CHUNK, 2 * CHUNK))  # <- chunks 8-15 ready
            for piece in range(2, 4):
                csl = slice(CHUNK * piece, CHUNK * (piece + 1))
                dma("ry0", csl)
                dma("ry1", csl)
                dma("ynb", csl)
            dma("ynb", slice(4 * CHUNK, NPAD))
            for piece in range(4):
                csl = slice(CHUNK * piece, CHUNK * (piece + 1))
                dma("rx0", csl)
                dma("rx1", csl)
                dma("xnb", csl)
            dma("xnb", slice(4 * CHUNK, NPAD))
            for k in ("lhs0", "lhs1", "laug"):
                dma(k, slice(half, 16 * P))
            dma("raugx")

            nb = {"x": t["xnb"], "y": t["ynb"]}

            # PE warm-up burst: ~10 throwaway matmuls on a zeroed tile run
            # while the first input DMAs land, so the HAM clock-gate is at
            # 2.4 GHz when the real chunks start.
            warm = consts.tile([P, BANK], b16, name="warm")
            nc.vector.memset(warm, 0.0)
            wpsum = psump.tile([P, CHUNK], f32, tag="psum", name="psum")
            for wi in range(10):
                nc.tensor.matmul(
                    wpsum[:, :BANK], warm[:, :P], warm,
                    start=(wi == 0), stop=(wi == 9),
                )

            # DVE-path chunks are paired: both stt results land in one wide
            # staging tile and ScalarE runs a single FD-4096 exp per pair
            # (amortizes the per-op overhead and the accumulator read).
            pend = {}  # open pair: tile, q0, n

            def flush_pair():
                if not pend:
                    return
                cols = pend["n"] * CHUNK
                q0 = pend["q0"]
                scr2 = scrp.tile([P, 2 * CHUNK], b16, tag="scr2", name="scr2")
                nc.scalar.activation(
                    out=scr2[:, :cols], in_=pend["tile"][:, :cols],
                    func=mybir.ActivationFunctionType.Exp,
                    scale=float(neg_c0),
                    accum_out=acc[:, q0 : q0 + 1],
                )
                pend.clear()

            for q, (kind, tt, role, start, _w, _m) in enumerate(chunk_list()):
                psum = psump.tile([P, CHUNK], f32)
                # PE-aug on the first kst batch (no ynb DMA dependency at
                # startup) and the sub16 specials (their partitions mix 16
                # row tiles, so no single per-partition norm exists).
                aug = kind == "sub16" or q < 8
                _emit_chunk_mms(
                    nc, t, psum, role, _chunk_jobs(kind, tt, start),
                    n_ki=3 if aug else 2,
                )
                if aug:
                    flush_pair()
                    scr2 = scrp.tile([P, 2 * CHUNK], b16, tag="scr2", name="scr2")
                    nc.scalar.activation(
                        out=scr2[:, :CHUNK], in_=psum,
                        func=mybir.ActivationFunctionType.Exp,
                        scale=float(neg_c0),
                        accum_out=acc[:, q : q + 1],
                    )
                    continue
                if pend and pend["w"] != _w:
                    flush_pair()
                if not pend:
                    pend.update(
                        tile=scrp.tile([P, 2 * CHUNK], f32, tag="tmpd", name="tmpd"),
                        q0=q, n=0, w=_w,
                    )
                half_sl = slice(pend["n"] * CHUNK, (pend["n"] + 1) * CHUNK)
                nc.vector.scalar_tensor_tensor(
                    out=pend["tile"][:, half_sl], in0=psum,
                    scalar=t["normb"][:, tt : tt + 1],
                    in1=nb[role][:, start : start + CHUNK],
                    op0=mybir.AluOpType.add, op1=mybir.AluOpType.add,
                )
                pend["n"] += 1
                if pend["n"] == 2:
                    flush_pair()
            flush_pair()
            nc.sync.dma_start(out=dio["acc"], in_=acc)

    nc.compile()
    return nc


def build_kernel(neg_cs, split=None):
    """General multi-bandwidth NEFF (fallback path)."""
    K = len(neg_cs)
    nc = bacc.Bacc("TRN2", debug=False, enable_asserts=False, num_devices=NCORES)
    f32, b16 = mybir.dt.float32, mybir.dt.bfloat16
    dio = _declare_io(nc, NCHUNKS * K)

    with tile.TileContext(nc) as tc:
        with (
            tc.tile_pool(name="consts", bufs=1) as consts,
            tc.tile_pool(name="scr", bufs=2) as scrp,
            tc.tile_pool(name="psum", bufs=2, space="PSUM") as psump,
        ):
            tiles = _alloc_and_dma(nc, consts, dio, NCHUNKS * K)
            acc = tiles["acc"]
            maskd = tiles["maskd"]

            GROUP = 2

            def emit_dve_chain(base_ap, cols, slot_j, slot_k):
                """Power chain on VectorE over base_ap [P, cols]:
                t4 = base^4 (sum -> slot_j), t16 = base^16 (sum -> slot_k)."""
                t2 = scrp.tile([P, GROUP * CHUNK], b16, tag="tmp", name="tmp")[:, :cols]
                t4 = scrp.tile([P, GROUP * CHUNK], b16, tag="t4", name="t4")[:, :cols]
                nc.vector.tensor_mul(t2, base_ap, base_ap)
                nc.vector.scalar_tensor_tensor(
                    out=t4, in0=t2, scalar=1.0, in1=t2,
                    op0=mybir.AluOpType.mult, op1=mybir.AluOpType.mult,
                    accum_out=slot_j,
                )
                if slot_k is not None:
                    t8 = scrp.tile([P, GROUP * CHUNK], b16, tag="tmp", name="tmp")[:, :cols]
                    scr = scrp.tile([P, GROUP * CHUNK], b16, tag="scr", name="scr")[:, :cols]
                    nc.vector.tensor_mul(t8, t4, t4)
                    nc.vector.scalar_tensor_tensor(
                        out=scr, in0=t8, scalar=1.0, in1=t8,
                        op0=mybir.AluOpType.mult, op1=mybir.AluOpType.mult,
                        accum_out=slot_k,
                    )

            chunks = chunk_list()
            grp = {}  # open base group: tile, q0, w, pk(bool), pkidx, n

            def flush_group(pj):
                if not grp:
                    return
                cols = grp["n"] * CHUNK
                q0 = grp["q0"]
                emit_dve_chain(
                    grp["tile"][:, :cols], cols,
                    acc[:, q0 * K + pj : q0 * K + pj + 1],
                    acc[:, q0 * K + grp["pkidx"] : q0 * K + grp["pkidx"] + 1]
                    if grp["pk"] else None,
                )
                grp.clear()

            for q, (kind, t, role, start, _w, mask) in enumerate(chunks):
                psum = psump.tile([P, CHUNK], f32)
                _emit_chunk_mms(nc, tiles, psum, role, _chunk_jobs(kind, t, start))
                if mask:
                    nc.vector.tensor_add(psum, psum, maskd)
                if split is None:
                    scr2 = scrp.tile([P, CHUNK], b16, tag="scr2")
                    for k, ncs in enumerate(neg_cs):
                        nc.scalar.activation(
                            out=scr2,
                            in_=psum,
                            func=mybir.ActivationFunctionType.Exp,
                            scale=float(ncs),
                            accum_out=acc[:, q * K + k : q * K + k + 1],
                        )
                    continue

                bi, pj, pk = split
                if q >= len(chunks) - 2 or kind == "sub16":
                    # tail rebalance: ACT takes c_k back so VectorE's
                    # trailing chains don't outlive the last ACT work
                    pk = None
                # Group up to GROUP same-weight mm chunks: their bases land
                # side by side in one wide tile and the DVE chain runs once
                # at FD GROUP*2048.  Accums go to the first chunk's slots;
                # the others stay zero (memset) so host weighting holds.
                if grp and not (
                    kind == "mm"
                    and grp["w"] == _w
                    and grp["pk"] == (pk is not None)
                    and grp["n"] < GROUP
                ):
                    flush_group(pj)
                if not grp:
                    grp.update(
                        tile=scrp.tile(
                            [P, GROUP * CHUNK], b16, tag="base", name="base"
                        ),
                        q0=q, w=_w, pk=(pk is not None), pkidx=pk, n=0,
                    )
                bsl = slice(grp["n"] * CHUNK, (grp["n"] + 1) * CHUNK)
                # base term first so DVE can overlap the rest
                nc.scalar.activation(
                    out=grp["tile"][:, bsl], in_=psum,
                    func=mybir.ActivationFunctionType.Exp,
                    scale=float(neg_cs[bi]),
                    accum_out=acc[:, q * K + bi : q * K + bi + 1],
                )
                grp["n"] += 1
                if grp["n"] == GROUP or kind != "mm":
                    flush_group(pj)
                scr2 = scrp.tile([P, CHUNK], b16, tag="scr2")
                for k, ncs in enumerate(neg_cs):
                    if k in (bi, pj, pk):
                        continue
                    nc.scalar.activation(
                        out=scr2, in_=psum,
                        func=mybir.ActivationFunctionType.Exp,
                        scale=float(ncs),
                        accum_out=acc[:, q * K + k : q * K + k + 1],
                    )

            if split is not None:
                flush_group(split[1])
            nc.sync.dma_start(out=dio["acc"], in_=acc)

    nc.compile()
    return nc


# ---------------------------------------------------------------- host


def _split_hi_lo(v64):
    hi = v64.astype(bf16)
    lo = (v64 - hi.astype(np.float64)).astype(bf16)
    return hi, lo


def _build_core_inputs(xT_b, yT_b, xnorm, ynorm, core):
    """Per-core input dict. xT_b/yT_b: [D, N] bf16; norms f64 [N]."""
    shift = P * (core + 1)
    rx = np.roll(xT_b, -shift, axis=1)
    ry = np.roll(yT_b, -shift, axis=1)
    ones = np.ones(N, bf16)
    xh, xl = _split_hi_lo(np.roll(xnorm, -shift))
    yh, yl = _split_hi_lo(np.roll(ynorm, -shift))
    raugx = np.stack([ones, ones, xh, xl])
    raugy = np.stack([ones, ones, yh, yl])

    lhs = np.empty((D, 16 * P), bf16)
    laug = np.empty((4, 16 * P), bf16)
    one128 = np.ones(P, bf16)
    for t in range(16):
        r = 8 * (t % 8) + core
        rows = slice(P * r, P * r + P)
        src = xT_b if t < 8 else yT_b
        nsrc = xnorm if t < 8 else ynorm
        lhs[:, P * t : P * (t + 1)] = (
            -2.0 * src[:, rows].astype(np.float32)
        ).astype(bf16)
        nh, nl = _split_hi_lo(nsrc[rows])
        laug[:, P * t : P * (t + 1)] = np.stack([nh, nl, one128, one128])

    mask = np.zeros((P, CHUNK), bf16)
    for s in range(16):
        mask[np.arange(P), P * s + np.arange(P)] = bf16(BIG)

    # broadcast norm rows (rolled like rx/ry, circularly padded) for the
    # VectorE norm-add path, and per-partition lhs-tile norms
    def bcast(norm64):
        row = np.roll(norm64, -shift).astype(bf16)
        row = np.concatenate([row, row[:CHUNK]])
        return np.ascontiguousarray(np.broadcast_to(row[None, :], (P, NPAD)))

    normb = np.empty((P, 16), np.float32)
    for tt in range(16):
        r = 8 * (tt % 8) + core
        nsrc = xnorm if tt < 8 else ynorm
        normb[:, tt] = nsrc[P * r : P * r + P].astype(np.float32)

    return {
        "xnb": bcast(xnorm),
        "ynb": bcast(ynorm),
        "normb": normb,
        "lhs0": np.ascontiguousarray(lhs[:P]),
        "lhs1": np.ascontiguousarray(lhs[P:]),
        "laug": np.ascontiguousarray(laug),
        "rx0": np.ascontiguousarray(rx[:P]),
        "rx1": np.ascontiguousarray(rx[P:]),
        "ry0": np.ascontiguousarray(ry[:P]),
        "ry1": np.ascontiguousarray(ry[P:]),
        "raugx": np.ascontiguousarray(raugx),
        "raugy": np.ascontiguousarray(raugy),
        "maskd": mask,
    }


_NC_CACHE = {}
_WARM = [False]


def _warmup():
    """Run a trivial NEFF once per process: the first NEFF execution in
    an axon session pays ~95 us of ring/queue init that would otherwise
    land inside the measured kernel."""
    if _WARM[0]:
        return
    nc = bacc.Bacc("TRN2", debug=False, enable_asserts=False, num_devices=NCORES)
    f32 = mybir.dt.float32
    d_in = nc.dram_tensor("wx", [P, P], f32, kind="ExternalInput").ap()
    d_out = nc.dram_tensor("wy", [P, P], f32, kind="ExternalOutput").ap()
    with tile.TileContext(nc) as tc:
        with tc.tile_pool(name="pool", bufs=1) as pool:
            t = pool.tile([P, P], f32)
            nc.sync.dma_start(out=t, in_=d_in)
            nc.sync.dma_start(out=d_out, in_=t)
    nc.compile()
    x = np.zeros((P, P), np.float32)
    for attempt in range(3):
        try:
            run_bass_kernel_spmd(
                nc, [{"wx": x}] * NCORES, core_ids=list(range(NCORES))
            )
            break
        except Exception:
            if attempt == 2:
                raise
            import time

            time.sleep(10)
    _WARM[0] = True


def _run_neff(nc, in_maps, trace):
    _warmup()
    for attempt in range(3):
        try:
            return run_bass_kernel_spmd(
                nc, in_maps, core_ids=list(range(NCORES)), trace=trace
            )
        except Exception:
            # transient device wedge (NRT_EXEC_UNIT_UNRECOVERABLE) clears
            # on a subsequent attempt; give it a moment and retry
            if attempt == 2:
                raise
            import time

            time.sleep(15)


def _prep_inputs(x, y):
    xT_b = np.ascontiguousarray(x.T).astype(bf16)
    yT_b = np.ascontiguousarray(y.T).astype(bf16)
    xnorm = (x.astype(np.float64) ** 2).sum(1)
    ynorm = (y.astype(np.float64) ** 2).sum(1)
    return [
        _build_core_inputs(xT_b, yT_b, xnorm, ynorm, c) for c in range(NCORES)
    ]


def _run_general(in_maps, cs, trace, use_split=True):
    K = len(cs)
    neg_cs = [-float(c) for c in cs]
    split = pick_split([float(c) for c in cs]) if use_split else None
    key = ("gen", tuple(float(v) for v in neg_cs), split)
    if key not in _NC_CACHE:
        _NC_CACHE[key] = build_kernel(neg_cs, split=split)
    gmaps = [{k: m[k] for k in GENERAL_KEYS} for m in in_maps]
    res = _run_neff(_NC_CACHE[key], gmaps, trace)

    weights = np.array([w for (_, _, _, _, w, _) in chunk_list()], np.float64)
    total = 0.0
    for core in range(NCORES):
        a = res.results[core]["acc"].astype(np.float64)  # [P, NCHUNKS*K]
        per_chunk = a.sum(0).reshape(NCHUNKS, K).sum(1)
        total += float(per_chunk @ weights)
    total += 2.0 * N * K  # analytic masked diagonals of kss + ktt
    out = np.float32(total / (float(N) * float(N) * K))
    return np.array(out, dtype=np.float32), res


def _run(source_features, target_features, bandwidths, trace=False, use_split=True):
    x = np.asarray(source_features, np.float32)
    y = np.asarray(target_features, np.float32)
    b = np.asarray(bandwidths, np.float64)
    cs = 1.0 / (2.0 * b * b)
    K = len(cs)

    in_maps = _prep_inputs(x, y)

    i0 = int(np.argmin(cs))
    c0 = float(cs[i0])
    xn_max = float(max(np.abs(x).max(initial=0.0), np.abs(y).max(initial=0.0)))
    # fast path assumptions: positive finite c0, feature scale where the
    # device's bf16/fp8 arithmetic keeps the diagonal near exp(0)=1
    if not np.isfinite(c0) or c0 <= 0.0 or xn_max > 100.0 or c0 > 1.0:
        return _run_general(in_maps, cs, trace, use_split)

    key = ("fast", c0)
    if key not in _NC_CACHE:
        _NC_CACHE[key] = build_fast_kernel(-c0)
    fmaps = [{k: m[k] for k in FAST_KEYS} for m in in_maps]
    res = _run_neff(_NC_CACHE[key], fmaps, trace)

    weights = np.array([w for (_, _, _, _, w, _) in chunk_list()], np.float64)
    absw = np.abs(weights)
    off0 = 0.0
    accs = []
    for core in range(NCORES):
        a = np.clip(res.results[core]["acc"].astype(np.float64), 0.0, None)
        accs.append(a)                         # [P, NCHUNKS] row-chunk sums
        off0 += float(a.sum(0) @ weights)
    # the diagonal-block chunk is unmasked: remove its 2N near-unit
    # diagonal terms (the analytic 2NK below re-adds exp(0) exactly)
    off0 -= 2.0 * N

    # Other bandwidths: exp(-c_k d) = a^(c_k/c0) elementwise with
    # a = exp(-c0 d) in [0, inf).  For p = c_k/c0 >= 1,
    # sum_j a_j^p <= (sum_j a_j)^p per (row, chunk) [lp subset l1], so the
    # stored partial sums bound each skipped term rigorously.  In the
    # diagonal chunk each row holds 16 diagonal a ~ 1 terms (one per
    # subtile, within 0.01 of 1 given the xn_max guard); subtract
    # 16*0.99 before powering its rows.
    total = off0
    bound = 0.0
    ok = True
    for k in range(K):
        if k == i0:
            continue
        pk = float(cs[k]) / c0
        if abs(pk - 1.0) < 1e-9:
            total += off0
        elif pk > 1.0:
            for a in accs:
                adj = a.copy()
                adj[:, DIAG_Q] = np.clip(adj[:, DIAG_Q] - 16 * 0.99, 0.0, None)
                bound += float((adj ** pk).sum(0) @ absw)
        else:
            ok = False
            break

    total += 2.0 * N * K  # analytic diagonals of kss + ktt, all k
    denom = float(N) * float(N) * K
    out_val = total / denom
    if not ok or bound / denom > max(1e-8, 1e-3 * abs(out_val)):
        return _run_general(in_maps, cs, trace, use_split)
    return np.array(np.float32(out_val), dtype=np.float32), res


def kernel(source_features, target_features, bandwidths):
    out, _ = _run(source_features, target_features, bandwidths)
    return out


# revision 27
# speedup vs baseline: 1.0590x; 1.0590x over previous
"""MMD loss kernel for Trainium2 (8 NeuronCores, Bass/Tile).

Math: out = mean_k mean_ij exp(-c_k * ||x_i - x_j||^2)          (kss)
          + same for y                                          (ktt)
          - 2 * same for (x, y)                                 (kst)
      with c_k = 1/(2 b_k^2), x: [8192, 256], y: [8192, 256].

Device strategy (identical SPMD program on 8 cores, different data):
  * PE computes the pairwise squared distances directly via feature
    augmentation: dist = [-2x; nh; nl; 1; 1]^T . [y; 1; 1; nh; nl]
    in bf16 (fp32 PSUM accumulate), K = 256 + 4.
  * ScalarE evaluates exp(-c_k * d) straight from PSUM in [128, 2048]
    chunks with fused accum_out row-sums (the mean reduction is free).
  * kss/ktt use a symmetric band decomposition: each 128-row tile r
    covers col tiles r+1..r+32 (mod 64) with weight 2, a d=32 batch
    with weight -1 removes the double count, and the diagonal subtiles
    (weight +1) have their exact diagonal masked to +1e30 (exp -> 0);
    the true diagonal contribution (N*K per matrix) is added on the
    host analytically.  This removes 1/3 of the exp work.
  * Per-core work: row tiles {8j + core}.  A per-core column rotation
    by 128*(core+1) makes every access offset core-independent, so one
    NEFF serves all 8 cores.

Fast path: the kernel-mean sum for bandwidth k is Sigma exp(-c_k d).
For a_j = exp(-c0 d_j) >= 0 and p = c_k/c0 >= 1, the lp-in-l1 norm
inequality gives Sigma a_j^p <= (Sigma a_j)^p per (row, chunk).  So the
device only evaluates exp for c0 = min_k c_k (one ACT pass per chunk);
the host bounds every other bandwidth's off-diagonal contribution from
the per-row-chunk partial sums the kernel already produces.  When the
bound is not negligible (never for gaussian-scale data, where larger
c_k underflow fp32 anyway) it falls back to the general multi-exp
kernel, so the result is correct for arbitrary inputs.
"""

import os
import numpy as np
import ml_dtypes

import concourse.bass as bass
import concourse.mybir as mybir
import concourse.tile as tile
from concourse import bacc
from concourse.bass_utils import run_bass_kernel_spmd

bf16 = ml_dtypes.bfloat16

N, D, P = 8192, 256, 128
NCORES, JPC = 8, 8          # 64 row tiles of 128, 8 per core
CHUNK = 2048                # PSUM chunk (4 banks) / ACT free dim
BANK = 512
NT = N // P                 # 64 subtile columns
BIG = np.float32(1e30)

# ---------------------------------------------------------------- job list


def chunk_list():
    """Chunk descriptors, identical on every core.

    (kind, lhs_tile, rhs_role, rhs_start, weight)
      kind: 'mm' (12-matmul streaming chunk) or 'sub16' (16 subtiles)
    """
    chunks = []
    # kst column-major: the 8 jobs of column piece cb only need that piece
    # of ry, so compute starts as soon as the first ~1 MB of DMA lands.
    for cb in range(4):
        for j in range(JPC):                  # kst, weight -2
            chunks.append(("mm", j, "y", cb * CHUNK, -2.0, False))
    for j in range(JPC):                      # kss band, weight +2
        for cb in range(2):
            chunks.append(("mm", j, "x", (1024 * j + CHUNK * cb) % N, 2.0, False))
    for j in range(JPC):                      # ktt band, weight +2
        for cb in range(2):
            chunks.append(("mm", 8 + j, "y", (1024 * j + CHUNK * cb) % N, 2.0, False))
    # the sub16 specials run last: their PE-heavy weight-load chains hide
    # under the tail of the VectorE/ScalarE pipeline
    chunks.append(("sub16", None, None, "d32", -1.0, False))   # d=32 fix
    chunks.append(("sub16", None, None, "diag", 1.0, True))    # diagonal
    return chunks


def sub16_layout(batch):
    """16 (lhs_tile, role, rhs_start) triples for a sub16 chunk."""
    out = []
    for s in range(16):
        jj = s % 8
        role = "x" if s < 8 else "y"
        if batch == "d32":
            st = (1024 * jj + 3968) % N
        else:
            st = (1024 * jj - 128) % N
        out.append((s, role, st))
    return out


NCHUNKS = len(chunk_list())  # 66

# ---------------------------------------------------------------- device


def pick_split(cs):
    """Find power-of-4 chains so some exp terms move to VectorE.

    Returns (base_idx, pow4_idx, pow16_idx|None) or None.  For the
    canonical bandwidths [0.1, 0.5, 1, 2, 5] -> cs = [50, 2, .5, .125,
    .02]: base c=0.125 (b=2), offloaded c=0.5 = base^4 and c=2 = base^16.
    """
    K = len(cs)

    def near(a, b):
        return abs(a - b) <= 1e-6 * abs(b)

    best = None
    for i in range(K):
        for j in range(K):
            if i == j or not near(cs[j], 4.0 * cs[i]):
                continue
            if best is None:
                best = (i, j, None)
            for k in range(K):
                if k not in (i, j) and near(cs[k], 4.0 * cs[j]):
                    return (i, j, k)
    return best


GENERAL_KEYS = (
    "lhs0", "lhs1", "laug", "rx0", "rx1", "ry0", "ry1",
    "raugx", "raugy", "maskd",
)
FAST_KEYS = (
    "lhs0", "lhs1", "laug", "rx0", "rx1", "ry0", "ry1",
    "raugx", "raugy", "xnb", "ynb", "normb",
)
NPAD = N + CHUNK  # broadcast norm rows padded for wrapping windows
DIAG_Q = 65  # chunk index of the unmasked diagonal-block chunk


def _declare_io(nc, acc_cols):
    f32, b16 = mybir.dt.float32, mybir.dt.bfloat16
    d = {}
    d["lhs0"] = nc.dram_tensor("lhs0", [P, 16 * P], b16, kind="ExternalInput").ap()
    d["lhs1"] = nc.dram_tensor("lhs1", [P, 16 * P], b16, kind="ExternalInput").ap()
    d["laug"] = nc.dram_tensor("laug", [4, 16 * P], b16, kind="ExternalInput").ap()
    d["rx0"] = nc.dram_tensor("rx0", [P, N], b16, kind="ExternalInput").ap()
    d["rx1"] = nc.dram_tensor("rx1", [P, N], b16, kind="ExternalInput").ap()
    d["ry0"] = nc.dram_tensor("ry0", [P, N], b16, kind="ExternalInput").ap()
    d["ry1"] = nc.dram_tensor("ry1", [P, N], b16, kind="ExternalInput").ap()
    d["raugx"] = nc.dram_tensor("raugx", [4, N], b16, kind="ExternalInput").ap()
    d["raugy"] = nc.dram_tensor("raugy", [4, N], b16, kind="ExternalInput").ap()
    d["maskd"] = nc.dram_tensor("maskd", [P, CHUNK], b16, kind="ExternalInput").ap()
    d["acc"] = nc.dram_tensor("acc", [P, acc_cols], f32, kind="ExternalOutput").ap()
    return d


def _alloc_and_dma(nc, consts, dio, acc_cols):
    """Allocate SBUF const tiles and issue the input DMAs in the order
    the chunk stream consumes them (kst piece 0 first)."""
    f32, b16 = mybir.dt.float32, mybir.dt.bfloat16
    t = {}
    t["lhs0"] = consts.tile([P, 16 * P], b16, name="lhs0")
    t["lhs1"] = consts.tile([P, 16 * P], b16, name="lhs1")
    t["laug"] = consts.tile([4, 16 * P], b16, name="laug")
    t["rx0"] = consts.tile([P, N], b16, name="rx0")
    t["rx1"] = consts.tile([P, N], b16, name="rx1")
    t["ry0"] = consts.tile([P, N], b16, name="ry0")
    t["ry1"] = consts.tile([P, N], b16, name="ry1")
    t["raugx"] = consts.tile([4, N], b16, name="raugx")
    t["raugy"] = consts.tile([4, N], b16, name="raugy")
    t["maskd"] = consts.tile([P, CHUNK], b16, name="maskd")
    t["acc"] = consts.tile([P, acc_cols], f32, name="acc")

    nc.vector.memset(t["acc"], 0.0)
    half = 8 * P
    for k in ("lhs0", "lhs1", "laug"):
        nc.sync.dma_start(out=t[k][:, :half], in_=dio[k][:, :half])
    nc.sync.dma_start(out=t["raugy"], in_=dio["raugy"])
    for k in ("lhs0", "lhs1", "laug"):
        nc.sync.dma_start(out=t[k][:, half:], in_=dio[k][:, half:])
    nc.sync.dma_start(out=t["raugx"], in_=dio["raugx"])
    for piece in range(4):
        csl = slice(CHUNK * piece, CHUNK * (piece + 1))
        for k in ("ry0", "ry1"):
            nc.sync.dma_start(out=t[k][:, csl], in_=dio[k][:, csl])
    for piece in range(4):
        csl = slice(CHUNK * piece, CHUNK * (piece + 1))
        for k in ("rx0", "rx1"):
            nc.sync.dma_start(out=t[k][:, csl], in_=dio[k][:, csl])
    nc.sync.dma_start(out=t["maskd"], in_=dio["maskd"])
    return t


def _chunk_jobs(kind, t, start):
    if kind == "mm":
        return [
            (BANK * b, BANK, t, None, (start + BANK * b) % N)
            for b in range(4)
        ]
    return [
        (P * s16, P, s16, role2, st2)
        for (s16, role2, st2) in sub16_layout(start)
    ]


def _emit_chunk_mms(nc, tiles, psum, role, jobs, n_ki=3):
    """jobs: list of (pcol, width, lhs_tile, role_override, rhs_start).
    k-outer / job-inner order so each lhsT loads once per contraction
    slice instead of once per bank.  n_ki=2 skips the augmentation pass
    (norms added later on VectorE instead)."""
    rmain = {"x": (tiles["rx0"], tiles["rx1"]), "y": (tiles["ry0"], tiles["ry1"])}
    raug_t = {"x": tiles["raugx"], "y": tiles["raugy"]}
    for ki in range(n_ki):
        for (pcol, width, t, role_ov, start) in jobs:
            r = role_ov or role
            m0, m1 = rmain[r]
            lsl = slice(P * t, P * t + P)
            if ki == 0:
                l, rr = tiles["lhs0"][:, lsl], m0[:, start : start + width]
            elif ki == 1:
                l, rr = tiles["lhs1"][:, lsl], m1[:, start : start + width]
            else:
                l, rr = tiles["laug"][:, lsl], raug_t[r][:, start : start + width]
            # start=True clears the has_written bits of the WHOLE psum
            # bank, so only the first matmul touching each bank carries it
            # (sub16 jobs pack four 128-col subtiles per 512-col bank).
            nc.tensor.matmul(
                psum[:, pcol : pcol + width], l, rr,
                start=(ki == 0 and pcol % BANK == 0), stop=(ki == n_ki - 1),
            )


def _declare_io_fast(nc):
    f32, b16 = mybir.dt.float32, mybir.dt.bfloat16
    d = {}
    d["lhs0"] = nc.dram_tensor("lhs0", [P, 16 * P], b16, kind="ExternalInput").ap()
    d["lhs1"] = nc.dram_tensor("lhs1", [P, 16 * P], b16, kind="ExternalInput").ap()
    d["laug"] = nc.dram_tensor("laug", [4, 16 * P], b16, kind="ExternalInput").ap()
    d["rx0"] = nc.dram_tensor("rx0", [P, N], b16, kind="ExternalInput").ap()
    d["rx1"] = nc.dram_tensor("rx1", [P, N], b16, kind="ExternalInput").ap()
    d["ry0"] = nc.dram_tensor("ry0", [P, N], b16, kind="ExternalInput").ap()
    d["ry1"] = nc.dram_tensor("ry1", [P, N], b16, kind="ExternalInput").ap()
    d["raugx"] = nc.dram_tensor("raugx", [4, N], b16, kind="ExternalInput").ap()
    d["raugy"] = nc.dram_tensor("raugy", [4, N], b16, kind="ExternalInput").ap()
    d["xnb"] = nc.dram_tensor("xnb", [P, NPAD], b16, kind="ExternalInput").ap()
    d["ynb"] = nc.dram_tensor("ynb", [P, NPAD], b16, kind="ExternalInput").ap()
    d["normb"] = nc.dram_tensor("normb", [P, 16], f32, kind="ExternalInput").ap()
    d["acc"] = nc.dram_tensor("acc", [P, NCHUNKS], f32, kind="ExternalOutput").ap()
    return d


def build_fast_kernel(neg_c0):
    """Single-bandwidth NEFF.  Per [128, 2048] chunk (pipelined):
      * PE: two bf16 128-deep contraction passes (8 matmuls, ~1.8us);
        the first kst batch and the sub16 specials also run the rank-4
        norm augmentation pass on PE (12 matmuls).
      * VectorE (most chunks): one scalar_tensor_tensor adds both norms,
        (psum + xn_p) + ynb_j -> f32 staging tile, ~2.3us.
      * ScalarE: ONE exp with fused accum_out row-sum into acc[:, q],
        ~2.1us.
    The diagonal subtiles are computed unmasked (d_ii ~ 0 -> exp ~ 1);
    the host subtracts the 2N near-unit diagonal terms analytically."""
    nc = bacc.Bacc("TRN2", debug=False, enable_asserts=False, num_devices=NCORES)
    f32, b16 = mybir.dt.float32, mybir.dt.bfloat16
    dio = _declare_io_fast(nc)

    with tile.TileContext(nc) as tc:
        with (
            tc.tile_pool(name="consts", bufs=1) as consts,
            tc.tile_pool(name="scr", bufs=2) as scrp,
            tc.tile_pool(name="psum", bufs=2, space="PSUM") as psump,
        ):
            t = {}
            t["lhs0"] = consts.tile([P, 16 * P], b16, name="lhs0")
            t["lhs1"] = consts.tile([P, 16 * P], b16, name="lhs1")
            t["laug"] = consts.tile([4, 16 * P], b16, name="laug")
            t["rx0"] = consts.tile([P, N], b16, name="rx0")
            t["rx1"] = consts.tile([P, N], b16, name="rx1")
            t["ry0"] = consts.tile([P, N], b16, name="ry0")
            t["ry1"] = consts.tile([P, N], b16, name="ry1")
            t["raugx"] = consts.tile([4, N], b16, name="raugx")
            t["raugy"] = consts.tile([4, N], b16, name="raugy")
            t["xnb"] = consts.tile([P, NPAD], b16, name="xnb")
            t["ynb"] = consts.tile([P, NPAD], b16, name="ynb")
            t["normb"] = consts.tile([P, 16], f32, name="normb")
            acc = consts.tile([P, NCHUNKS], f32, name="acc")
            nc.vector.memset(acc, 0.0)

            half = 8 * P

            def dma(key, sl=None):
                if sl is None:
                    nc.sync.dma_start(out=t[key], in_=dio[key])
                else:
                    nc.sync.dma_start(out=t[key][:, sl], in_=dio[key][:, sl])

            # DMA stream in chunk-consumption order
            for k in ("lhs0", "lhs1", "laug"):
                dma(k, slice(0, half))
            dma("raugy")
            dma("ry0", slice(0, CHUNK))
            dma("ry1", slice(0, CHUNK))          # <- chunks 0-3 ready
            dma("normb")
            dma("ynb", slice(0, CHUNK))          # <- chunks 4-7 ready
            dma("ry0", slice(CHUNK, 2 * CHUNK))
            dma("ry1", slice(CHUNK, 2 * CHUNK))
            dma("ynb", slice(CHUNK, 2 * CHUNK))  # <- chunks 8-15 ready
            for piece in range(2, 4):
                csl = slice(CHUNK * piece, CHUNK * (piece + 1))
                dma("ry0", csl)
                dma("ry1", csl)
                dma("ynb", csl)
            dma("ynb", slice(4 * CHUNK, NPAD))
            for piece in range(4):
                csl = slice(CHUNK * piece, CHUNK * (piece + 1))
                dma("rx0", csl)
                dma("rx1", csl)
                dma("xnb", csl)
            dma("xnb", slice(4 * CHUNK, NPAD))
            for k in ("lhs0", "lhs1", "laug"):
                dma(k, slice(half, 16 * P))
            dma("raugx")

            nb = {"x": t["xnb"], "y": t["ynb"]}

            # PE warm-up burst: ~10 throwaway matmuls on a zeroed tile run
            # while the first input DMAs land, so the HAM clock-gate is at
            # 2.4 GHz when the real chunks start.
            warm = consts.tile([P, BANK], b16, name="warm")
            nc.vector.memset(warm, 0.0)
            wpsum = psump.tile([P, CHUNK], f32, tag="psum", name="psum")
            for wi in range(6):
                nc.tensor.matmul(
                    wpsum[:, :BANK], warm[:, :P], warm,
                    start=(wi == 0), stop=(wi == 5),
                )

            # DVE-path chunks are paired: both stt results land in one wide
            # staging tile and ScalarE runs a single FD-4096 exp per pair
            # (amortizes the per-op overhead and the accumulator read).
            pend = {}  # open pair: tile, q0, n

            def flush_pair():
                if not pend:
                    return
                cols = pend["n"] * CHUNK
                q0 = pend["q0"]
                scr2 = scrp.tile([P, 2 * CHUNK], b16, tag="scr2", name="scr2")
                nc.scalar.activation(
                    out=scr2[:, :cols], in_=pend["tile"][:, :cols],
                    func=mybir.ActivationFunctionType.Exp,
                    scale=float(neg_c0),
                    accum_out=acc[:, q0 : q0 + 1],
                )
                pend.clear()

            for q, (kind, tt, role, start, _w, _m) in enumerate(chunk_list()):
                psum = psump.tile([P, CHUNK], f32)
                # PE-aug on the first kst chunks (no ynb DMA dependency at
                # startup) and the sub16 specials (their partitions mix 16
                # row tiles, so no single per-partition norm exists).
                aug = kind == "sub16" or q < 4
                _emit_chunk_mms(
                    nc, t, psum, role, _chunk_jobs(kind, tt, start),
                    n_ki=3 if aug else 2,
                )
                if aug:
                    flush_pair()
                    scr2 = scrp.tile([P, 2 * CHUNK], b16, tag="scr2", name="scr2")
                    nc.scalar.activation(
                        out=scr2[:, :CHUNK], in_=psum,
                        func=mybir.ActivationFunctionType.Exp,
                        scale=float(neg_c0),
                        accum_out=acc[:, q : q + 1],
                    )
                    continue
                if pend and pend["w"] != _w:
                    flush_pair()
                if not pend:
                    pend.update(
                        tile=scrp.tile([P, 2 * CHUNK], f32, tag="tmpd", name="tmpd"),
                        q0=q, n=0, w=_w,
                    )
                half_sl = slice(pend["n"] * CHUNK, (pend["n"] + 1) * CHUNK)
                nc.vector.scalar_tensor_tensor(
                    out=pend["tile"][:, half_sl], in0=psum,
                    scalar=t["normb"][:, tt : tt + 1],
                    in1=nb[role][:, start : start + CHUNK],
                    op0=mybir.AluOpType.add, op1=mybir.AluOpType.add,
                )
                pend["n"] += 1
                if pend["n"] == 2:
                    flush_pair()
            flush_pair()
            nc.sync.dma_start(out=dio["acc"], in_=acc)

    nc.compile()
    return nc


def build_kernel(neg_cs, split=None):
    """General multi-bandwidth NEFF (fallback path)."""
    K = len(neg_cs)
    nc = bacc.Bacc("TRN2", debug=False, enable_asserts=False, num_devices=NCORES)
    f32, b16 = mybir.dt.float32, mybir.dt.bfloat16
    dio = _declare_io(nc, NCHUNKS * K)

    with tile.TileContext(nc) as tc:
        with (
            tc.tile_pool(name="consts", bufs=1) as consts,
            tc.tile_pool(name="scr", bufs=2) as scrp,
            tc.tile_pool(name="psum", bufs=2, space="PSUM") as psump,
        ):
            tiles = _alloc_and_dma(nc, consts, dio, NCHUNKS * K)
            acc = tiles["acc"]
            maskd = tiles["maskd"]

            GROUP = 2

            def emit_dve_chain(base_ap, cols, slot_j, slot_k):
                """Power chain on VectorE over base_ap [P, cols]:
                t4 = base^4 (sum -> slot_j), t16 = base^16 (sum -> slot_k)."""
                t2 = scrp.tile([P, GROUP * CHUNK], b16, tag="tmp", name="tmp")[:, :cols]
                t4 = scrp.tile([P, GROUP * CHUNK], b16, tag="t4", name="t4")[:, :cols]
                nc.vector.tensor_mul(t2, base_ap, base_ap)
                nc.vector.scalar_tensor_tensor(
                    out=t4, in0=t2, scalar=1.0, in1=t2,
                    op0=mybir.AluOpType.mult, op1=mybir.AluOpType.mult,
                    accum_out=slot_j,
                )
                if slot_k is not None:
                    t8 = scrp.tile([P, GROUP * CHUNK], b16, tag="tmp", name="tmp")[:, :cols]
                    scr = scrp.tile([P, GROUP * CHUNK], b16, tag="scr", name="scr")[:, :cols]
                    nc.vector.tensor_mul(t8, t4, t4)
                    nc.vector.scalar_tensor_tensor(
                        out=scr, in0=t8, scalar=1.0, in1=t8,
                        op0=mybir.AluOpType.mult, op1=mybir.AluOpType.mult,
                        accum_out=slot_k,
                    )

            chunks = chunk_list()
            grp = {}  # open base group: tile, q0, w, pk(bool), pkidx, n

            def flush_group(pj):
                if not grp:
                    return
                cols = grp["n"] * CHUNK
                q0 = grp["q0"]
                emit_dve_chain(
                    grp["tile"][:, :cols], cols,
                    acc[:, q0 * K + pj : q0 * K + pj + 1],
                    acc[:, q0 * K + grp["pkidx"] : q0 * K + grp["pkidx"] + 1]
                    if grp["pk"] else None,
                )
                grp.clear()

            for q, (kind, t, role, start, _w, mask) in enumerate(chunks):
                psum = psump.tile([P, CHUNK], f32)
                _emit_chunk_mms(nc, tiles, psum, role, _chunk_jobs(kind, t, start))
                if mask:
                    nc.vector.tensor_add(psum, psum, maskd)
                if split is None:
                    scr2 = scrp.tile([P, CHUNK], b16, tag="scr2")
                    for k, ncs in enumerate(neg_cs):
                        nc.scalar.activation(
                            out=scr2,
                            in_=psum,
                            func=mybir.ActivationFunctionType.Exp,
                            scale=float(ncs),
                            accum_out=acc[:, q * K + k : q * K + k + 1],
                        )
                    continue

                bi, pj, pk = split
                if q >= len(chunks) - 2 or kind == "sub16":
                    # tail rebalance: ACT takes c_k back so VectorE's
                    # trailing chains don't outlive the last ACT work
                    pk = None
                # Group up to GROUP same-weight mm chunks: their bases land
                # side by side in one wide tile and the DVE chain runs once
                # at FD GROUP*2048.  Accums go to the first chunk's slots;
                # the others stay zero (memset) so host weighting holds.
                if grp and not (
                    kind == "mm"
                    and grp["w"] == _w
                    and grp["pk"] == (pk is not None)
                    and grp["n"] < GROUP
                ):
                    flush_group(pj)
                if not grp:
                    grp.update(
                        tile=scrp.tile(
                            [P, GROUP * CHUNK], b16, tag="base", name="base"
                        ),
                        q0=q, w=_w, pk=(pk is not None), pkidx=pk, n=0,
                    )
                bsl = slice(grp["n"] * CHUNK, (grp["n"] + 1) * CHUNK)
                # base term first so DVE can overlap the rest
                nc.scalar.activation(
                    out=grp["tile"][:, bsl], in_=psum,
                    func=mybir.ActivationFunctionType.Exp,
                    scale=float(neg_cs[bi]),
                    accum_out=acc[:, q * K + bi : q * K + bi + 1],
                )
                grp["n"] += 1
                if grp["n"] == GROUP or kind != "mm":
                    flush_group(pj)
                scr2 = scrp.tile([P, CHUNK], b16, tag="scr2")
                for k, ncs in enumerate(neg_cs):
                    if k in (bi, pj, pk):
                        continue
                    nc.scalar.activation(
                        out=scr2, in_=psum,
                        func=mybir.ActivationFunctionType.Exp,
                        scale=float(ncs),
                        accum_out=acc[:, q * K + k : q * K + k + 1],
                    )

            if split is not None:
                flush_group(split[1])
            nc.sync.dma_start(out=dio["acc"], in_=acc)

    nc.compile()
    return nc


# ---------------------------------------------------------------- host


def _split_hi_lo(v64):
    hi = v64.astype(bf16)
    lo = (v64 - hi.astype(np.float64)).astype(bf16)
    return hi, lo


def _build_core_inputs(xT_b, yT_b, xnorm, ynorm, core):
    """Per-core input dict. xT_b/yT_b: [D, N] bf16; norms f64 [N]."""
    shift = P * (core + 1)
    rx = np.roll(xT_b, -shift, axis=1)
    ry = np.roll(yT_b, -shift, axis=1)
    ones = np.ones(N, bf16)
    xh, xl = _split_hi_lo(np.roll(xnorm, -shift))
    yh, yl = _split_hi_lo(np.roll(ynorm, -shift))
    raugx = np.stack([ones, ones, xh, xl])
    raugy = np.stack([ones, ones, yh, yl])

    lhs = np.empty((D, 16 * P), bf16)
    laug = np.empty((4, 16 * P), bf16)
    one128 = np.ones(P, bf16)
    for t in range(16):
        r = 8 * (t % 8) + core
        rows = slice(P * r, P * r + P)
        src = xT_b if t < 8 else yT_b
        nsrc = xnorm if t < 8 else ynorm
        lhs[:, P * t : P * (t + 1)] = (
            -2.0 * src[:, rows].astype(np.float32)
        ).astype(bf16)
        nh, nl = _split_hi_lo(nsrc[rows])
        laug[:, P * t : P * (t + 1)] = np.stack([nh, nl, one128, one128])

    mask = np.zeros((P, CHUNK), bf16)
    for s in range(16):
        mask[np.arange(P), P * s + np.arange(P)] = bf16(BIG)

    # broadcast norm rows (rolled like rx/ry, circularly padded) for the
    # VectorE norm-add path, and per-partition lhs-tile norms
    def bcast(norm64):
        row = np.roll(norm64, -shift).astype(bf16)
        row = np.concatenate([row, row[:CHUNK]])
        return np.ascontiguousarray(np.broadcast_to(row[None, :], (P, NPAD)))

    normb = np.empty((P, 16), np.float32)
    for tt in range(16):
        r = 8 * (tt % 8) + core
        nsrc = xnorm if tt < 8 else ynorm
        normb[:, tt] = nsrc[P * r : P * r + P].astype(np.float32)

    return {
        "xnb": bcast(xnorm),
        "ynb": bcast(ynorm),
        "normb": normb,
        "lhs0": np.ascontiguousarray(lhs[:P]),
        "lhs1": np.ascontiguousarray(lhs[P:]),
        "laug": np.ascontiguousarray(laug),
        "rx0": np.ascontiguousarray(rx[:P]),
        "rx1": np.ascontiguousarray(rx[P:]),
        "ry0": np.ascontiguousarray(ry[:P]),
        "ry1": np.ascontiguousarray(ry[P:]),
        "raugx": np.ascontiguousarray(raugx),
        "raugy": np.ascontiguousarray(raugy),
        "maskd": mask,
    }


_NC_CACHE = {}
_WARM = [False]


def _warmup():
    """Run a trivial NEFF once per process: the first NEFF execution in
    an axon session pays ~95 us of ring/queue init that would otherwise
    land inside the measured kernel."""
    if _WARM[0]:
        return
    nc = bacc.Bacc("TRN2", debug=False, enable_asserts=False, num_devices=NCORES)
    f32 = mybir.dt.float32
    d_in = nc.dram_tensor("wx", [P, P], f32, kind="ExternalInput").ap()
    d_out = nc.dram_tensor("wy", [P, P], f32, kind="ExternalOutput").ap()
    with tile.TileContext(nc) as tc:
        with tc.tile_pool(name="pool", bufs=1) as pool:
            t = pool.tile([P, P], f32)
            nc.sync.dma_start(out=t, in_=d_in)
            nc.sync.dma_start(out=d_out, in_=t)
    nc.compile()
    x = np.zeros((P, P), np.float32)
    for attempt in range(3):
        try:
            run_bass_kernel_spmd(
                nc, [{"wx": x}] * NCORES, core_ids=list(range(NCORES))
            )
            break
        except Exception:
            if attempt == 2:
                raise
            import time

            time.sleep(10)
    _WARM[0] = True


def _run_neff(nc, in_maps, trace):
    _warmup()
    for attempt in range(3):
        try:
            return run_bass_kernel_spmd(
                nc, in_maps, core_ids=list(range(NCORES)), trace=trace
            )
        except Exception:
            # transient device wedge (NRT_EXEC_UNIT_UNRECOVERABLE) clears
            # on a subsequent attempt; give it a moment and retry
            if attempt == 2:
                raise
            import time

            time.sleep(15)


def _prep_inputs(x, y):
    xT_b = np.ascontiguousarray(x.T).astype(bf16)
    yT_b = np.ascontiguousarray(y.T).astype(bf16)
    xnorm = (x.astype(np.float64) ** 2).sum(1)
    ynorm = (y.astype(np.float64) ** 2).sum(1)
    return [
        _build_core_inputs(xT_b, yT_b, xnorm, ynorm, c) for c in range(NCORES)
    ]


def _run_general(in_maps, cs, trace, use_split=True):
    K = len(cs)
    neg_cs = [-float(c) for c in cs]
    split = pick_split([float(c) for c in cs]) if use_split else None
    key = ("gen", tuple(float(v) for v in neg_cs), split)
    if key not in _NC_CACHE:
        _NC_CACHE[key] = build_kernel(neg_cs, split=split)
    gmaps = [{k: m[k] for k in GENERAL_KEYS} for m in in_maps]
    res = _run_neff(_NC_CACHE[key], gmaps, trace)

    weights = np.array([w for (_, _, _, _, w, _) in chunk_list()], np.float64)
    total = 0.0
    for core in range(NCORES):
        a = res.results[core]["acc"].astype(np.float64)  # [P, NCHUNKS*K]
        per_chunk = a.sum(0).reshape(NCHUNKS, K).sum(1)
        total += float(per_chunk @ weights)
    total += 2.0 * N * K  # analytic masked diagonals of kss + ktt
    out = np.float32(total / (float(N) * float(N) * K))
    return np.array(out, dtype=np.float32), res


def _run(source_features, target_features, bandwidths, trace=False, use_split=True):
    x = np.asarray(source_features, np.float32)
    y = np.asarray(target_features, np.float32)
    b = np.asarray(bandwidths, np.float64)
    cs = 1.0 / (2.0 * b * b)
    K = len(cs)

    in_maps = _prep_inputs(x, y)

    i0 = int(np.argmin(cs))
    c0 = float(cs[i0])
    xn_max = float(max(np.abs(x).max(initial=0.0), np.abs(y).max(initial=0.0)))
    # fast path assumptions: positive finite c0, feature scale where the
    # device's bf16/fp8 arithmetic keeps the diagonal near exp(0)=1
    if not np.isfinite(c0) or c0 <= 0.0 or xn_max > 100.0 or c0 > 1.0:
        return _run_general(in_maps, cs, trace, use_split)

    key = ("fast", c0)
    if key not in _NC_CACHE:
        _NC_CACHE[key] = build_fast_kernel(-c0)
    fmaps = [{k: m[k] for k in FAST_KEYS} for m in in_maps]
    res = _run_neff(_NC_CACHE[key], fmaps, trace)

    weights = np.array([w for (_, _, _, _, w, _) in chunk_list()], np.float64)
    absw = np.abs(weights)
    off0 = 0.0
    accs = []
    for core in range(NCORES):
        a = np.clip(res.results[core]["acc"].astype(np.float64), 0.0, None)
        accs.append(a)                         # [P, NCHUNKS] row-chunk sums
        off0 += float(a.sum(0) @ weights)
    # the diagonal-block chunk is unmasked: remove its 2N near-unit
    # diagonal terms (the analytic 2NK below re-adds exp(0) exactly)
    off0 -= 2.0 * N

    # Other bandwidths: exp(-c_k d) = a^(c_k/c0) elementwise with
    # a = exp(-c0 d) in [0, inf).  For p = c_k/c0 >= 1,
    # sum_j a_j^p <= (sum_j a_j)^p per (row, chunk) [lp subset l1], so the
    # stored partial sums bound each skipped term rigorously.  In the
    # diagonal chunk each row holds 16 diagonal a ~ 1 terms (one per
    # subtile, within 0.01 of 1 given the xn_max guard); subtract
    # 16*0.99 before powering its rows.
    total = off0
    bound = 0.0
    ok = True
    for k in range(K):
        if k == i0:
            continue
        pk = float(cs[k]) / c0
        if abs(pk - 1.0) < 1e-9:
            total += off0
        elif pk > 1.0:
            for a in accs:
                adj = a.copy()
                adj[:, DIAG_Q] = np.clip(adj[:, DIAG_Q] - 16 * 0.99, 0.0, None)
                bound += float((adj ** pk).sum(0) @ absw)
        else:
            ok = False
            break

    total += 2.0 * N * K  # analytic diagonals of kss + ktt, all k
    denom = float(N) * float(N) * K
    out_val = total / denom
    if not ok or bound / denom > max(1e-8, 1e-3 * abs(out_val)):
        return _run_general(in_maps, cs, trace, use_split)
    return np.array(np.float32(out_val), dtype=np.float32), res


def kernel(source_features, target_features, bandwidths):
    out, _ = _run(source_features, target_features, bandwidths)
    return out
